# revision 4
# baseline (speedup 1.0000x reference)
"""Enformer relative-position attention block on 8 Trainium2 NeuronCores, v2.

Sharding: core c handles batch b = c//4 and head pair hp = c%4 (heads 2hp,
2hp+1).  Weight slices per head pair; x sharded by batch.  Each core computes
a partial (n, dim) output; host sums 4 partials per batch and adds bo.

v2 changes vs baseline:
  - x is transposed on the HOST: kernel loads xT directly (no PE transposes).
  - content + rel-window logit matmuls are packed per head PAIR via K=64
    row tiling (heads at array rows 0-63 / 64-127 run concurrently).
  - attention probs p are transposed by ONE xbar DMA-transpose per (chunk,
    head) in bf16 (no PE transposes, no DVE copies).
  - row sums come free from an appended ones-column on the po2 stationary.
  - rel/pos/value/out-proj paths in bf16; content logits path stays f32r.
"""

import math

import numpy as np
import ml_dtypes

import concourse.bass as bass
import concourse.mybir as mybir
from concourse import bacc
from concourse.masks import make_identity
from concourse.tile import TileContext
from concourse.bass_utils import run_bass_kernel_spmd

F32 = mybir.dt.float32
F32R = mybir.dt.float32r
BF16 = mybir.dt.bfloat16

HEADS, DIM, DK, DV, NRPF = 8, 1536, 64, 192, 192
N = 1536
NCH = N // 128           # 12 query chunks
DIMCH = DIM // 128       # 12 contraction chunks
TWO_N1 = 2 * N - 1       # 3071
POSW = TWO_N1 + 1        # padded to even width 3072
WIN = 1664               # padded rel-window width (cols >1662 unused)
WSTR = WIN - 1           # diagonal read row stride
NSUP = 3                 # supers of 512 rows
SCALE = DK ** -0.5
PTW = 6272               # pT tile row pitch: 12*512 + 128 pad (avoids AP merge)


def _r(ap):
    return ap.bitcast(F32R)


def build_nc(reps=1):
    nc = bacc.Bacc(None)

    xt_d = nc.declare_dram_parameter("xt_b", [DIM, N], F32R, isOutput=False)
    wq_d = nc.declare_dram_parameter("wq_s", [DIM, 128], F32R, isOutput=False)
    wk_d = nc.declare_dram_parameter("wk_s", [DIM, 128], F32R, isOutput=False)
    wv_d = nc.declare_dram_parameter("wv_s", [DIM, 384], F32R, isOutput=False)
    wrk_d = nc.declare_dram_parameter("wrk_s", [NRPF, 128], BF16, isOutput=False)
    post_d = nc.declare_dram_parameter("posT", [NRPF, POSW], BF16, isOutput=False)
    bc_d = nc.declare_dram_parameter("bc_s", [128], F32, isOutput=False)
    bp_d = nc.declare_dram_parameter("bp_s", [128], F32, isOutput=False)
    wo_d = nc.declare_dram_parameter("wo_s", [384, DIM], BF16, isOutput=False)
    out_d = nc.declare_dram_parameter("out_p", [N, DIM], BF16, isOutput=True)

    with TileContext(nc) as tc:
      for _rep in range(reps):
        with tc.tile_pool(name="const", bufs=1) as const, \
             tc.tile_pool(name="persist", bufs=1) as persist:
            ident_f = const.tile([128, 128], F32, name="identf", tag="identf")
            ident_b = const.tile([128, 128], BF16, name="identb", tag="identb")
            bc_t = const.tile([128, 1], F32, name="bc", tag="bc")
            bp_t = const.tile([128, 1], F32, name="bp", tag="bp")
            nc.sync.dma_start(out=bc_t[:], in_=bc_d.rearrange("(p o) -> p o", o=1))
            nc.sync.dma_start(out=bp_t[:], in_=bp_d.rearrange("(p o) -> p o", o=1))

            # persistent packed tensors (rows 0-63 head0, 64-127 head1)
            qc_t = persist.tile([128, N], F32R, name="qc", tag="qc")
            qp_t = persist.tile([128, N], BF16, name="qp", tag="qp")
            k_t = persist.tile([128, N], F32R, name="k", tag="k")
            relk = persist.tile([128, POSW], BF16, name="relk", tag="relk")
            v_t = [persist.tile([128, 386], BF16, name=f"v{r}", tag=f"v{r}")
                   for r in range(NCH)]
            o1_h = [persist.tile([128, N], BF16, name=f"o1{h}", tag=f"o1{h}")
                    for h in range(2)]
            o2s = persist.tile([128, N], BF16, name="o2s", tag="o2s")

            for r in range(NCH):
                nc.vector.memset(v_t[r][:, 192:193], 1.0)
                nc.vector.memset(v_t[r][:, 385:386], 1.0)

            # ---------------- rel_k projection ----------------
            with tc.tile_pool(name="pos", bufs=1) as pos_pool, \
                 tc.tile_pool(name="ps_rk", bufs=2, space="PSUM") as ps_rk:
                pos_a = pos_pool.tile([128, POSW], BF16, name="posA", tag="posA")
                pos_b = pos_pool.tile([64, POSW], BF16, name="posB", tag="posB")
                wrk_a = pos_pool.tile([128, 128], BF16, name="wrkA", tag="wrkA")
                wrk_b = pos_pool.tile([64, 128], BF16, name="wrkB", tag="wrkB")
                nc.gpsimd.dma_start(out=pos_a[:], in_=post_d[0:128, :])
                nc.gpsimd.dma_start(out=pos_b[:], in_=post_d[128:NRPF, :])
                nc.gpsimd.dma_start(out=wrk_a[:], in_=wrk_d[0:128, :])
                nc.gpsimd.dma_start(out=wrk_b[:], in_=wrk_d[128:NRPF, :])
                for cb in range(6):
                    c0 = 512 * cb
                    ps = ps_rk.tile([128, 512], F32, name="psrk", tag="psrk")
                    nc.tensor.matmul(ps[:], wrk_a[:], pos_a[:, c0:c0 + 512],
                                     start=True, stop=False)
                    nc.tensor.matmul(ps[:], wrk_b[:], pos_b[:, c0:c0 + 512],
                                     start=False, stop=True)
                    nc.scalar.copy(relk[:, c0:c0 + 512], ps[:])

            # ---------------- q/k/v projections (from host-side xT) --------
            with tc.tile_pool(name="wqkv", bufs=1) as wqkv, \
                 tc.tile_pool(name="xts", bufs=2) as xts_pool, \
                 tc.tile_pool(name="ps_qk", bufs=2, space="PSUM") as ps_qk, \
                 tc.tile_pool(name="ps_v", bufs=2, space="PSUM") as ps_v:
                wq_t = wqkv.tile([128, DIMCH * 128], F32R, name="wq", tag="wq")
                wk_t = wqkv.tile([128, DIMCH * 128], F32R, name="wk", tag="wk")
                wv_t = wqkv.tile([128, DIMCH * 384], F32R, name="wv", tag="wv")
                for rr in range(DIMCH):
                    nc.scalar.dma_start(out=wq_t[:, 128 * rr:128 * rr + 128],
                                        in_=wq_d[128 * rr:128 * rr + 128, :])
                    nc.scalar.dma_start(out=wk_t[:, 128 * rr:128 * rr + 128],
                                        in_=wk_d[128 * rr:128 * rr + 128, :])
                    nc.scalar.dma_start(out=wv_t[:, 384 * rr:384 * rr + 384],
                                        in_=wv_d[128 * rr:128 * rr + 128, :])

                # identity built here so its ACT copy doesn't delay the
                # weight-load DMAs queued on the scalar ring at startup
                make_identity(nc, ident_f)
                nc.scalar.copy(ident_b[:], ident_f[:])

                for s in range(NSUP):
                    cs = slice(512 * s, 512 * s + 512)
                    xts = xts_pool.tile([128, DIMCH * 512], F32R, name="xts",
                                        tag="xts")
                    for rr in range(DIMCH):
                        nc.sync.dma_start(
                            out=xts[:, 512 * rr:512 * rr + 512],
                            in_=xt_d[128 * rr:128 * rr + 128,
                                     512 * s:512 * s + 512])
                    psq = ps_qk.tile([128, 512], F32, name="psq", tag="psq")
                    psk = ps_qk.tile([128, 512], F32, name="psk", tag="psk")
                    for rr in range(DIMCH):
                        nc.tensor.matmul(psq[:], wq_t[:, 128 * rr:128 * rr + 128],
                                         xts[:, 512 * rr:512 * rr + 512],
                                         start=(rr == 0), stop=(rr == DIMCH - 1))
                    for rr in range(DIMCH):
                        nc.tensor.matmul(psk[:], wk_t[:, 128 * rr:128 * rr + 128],
                                         xts[:, 512 * rr:512 * rr + 512],
                                         start=(rr == 0), stop=(rr == DIMCH - 1))
                    nc.scalar.activation(qc_t[:, cs], psq[:],
                                         mybir.ActivationFunctionType.Identity,
                                         bias=bc_t[:], scale=SCALE)
                    nc.scalar.activation(qp_t[:, cs], psq[:],
                                         mybir.ActivationFunctionType.Identity,
                                         bias=bp_t[:], scale=SCALE)
                    nc.vector.tensor_copy(k_t[:, cs], psk[:])
                    for g in range(4):
                        ci = 4 * s + g
                        psv = ps_v.tile([128, 384], F32, name="psv", tag="psv")
                        for rr in range(DIMCH):
                            nc.tensor.matmul(
                                psv[:],
                                xts[:, 512 * rr + 128 * g:512 * rr + 128 * g + 128],
                                wv_t[:, 384 * rr:384 * rr + 384],
                                start=(rr == 0), stop=(rr == DIMCH - 1))
                        nc.vector.tensor_copy(v_t[ci][:, 0:192], psv[:, 0:192])
                        nc.vector.tensor_copy(v_t[ci][:, 193:385], psv[:, 192:384])

            # ---------------- attention + interleaved output projection ----
            with tc.tile_pool(name="wwin", bufs=3) as w_pool, \
                 tc.tile_pool(name="rsh", bufs=3) as rsh_pool, \
                 tc.tile_pool(name="pbuf", bufs=2) as p_pool, \
                 tc.tile_pool(name="ptr", bufs=2) as pt_pool, \
                 tc.tile_pool(name="small", bufs=4) as small, \
                 tc.tile_pool(name="wo", bufs=1) as wo_pool, \
                 tc.tile_pool(name="osb", bufs=3) as osb_pool, \
                 tc.tile_pool(name="ps_w", bufs=2, space="PSUM") as ps_w, \
                 tc.tile_pool(name="ps_c", bufs=3, space="PSUM") as ps_c, \
                 tc.tile_pool(name="ps_o", bufs=1, space="PSUM") as ps_o:
                wo_t = [wo_pool.tile([128, DIM], BF16, name=f"wo{t}", tag=f"wo{t}")
                        for t in range(3)]
                for t, (r0, r1) in zip(wo_t, [(0, 128), (128, 256), (256, 384)]):
                    nc.scalar.dma_start(out=t[:], in_=wo_d[r0:r1, :])
                o_pieces = [o1_h[0], o1_h[1], o2s]
                po_tags = ["po1", "po2"]
                rsh_q = {}
                ptT_q = {}

                def wwin_stage(ci):
                    # rel window logits + shift; both heads row-tiled
                    i0 = 128 * ci
                    w0 = (N - 1) - i0 - 127
                    we = [w_pool.tile([128, WIN], BF16, name=f"we{h}",
                                      tag=f"we{h}") for h in range(2)]
                    for c0 in (0, 416, 832, 1248):
                        pw = [ps_w.tile([128, 416], F32, name="pw", tag="pw")
                              for _ in range(2)]
                        for h in range(2):
                            hs = slice(64 * h, 64 * h + 64)
                            nc.tensor.matmul(
                                pw[h][:], qp_t[hs, i0:i0 + 128],
                                relk[hs, w0 + c0:w0 + c0 + 416],
                                start=True, stop=True)
                        nc.vector.tensor_copy(we[0][:, c0:c0 + 416], pw[0][:])
                        nc.vector.tensor_copy(we[1][:, c0:c0 + 416], pw[1][:])
                    rsh = [rsh_pool.tile([128, N], BF16, name=f"rsh{h}",
                                         tag=f"rsh{h}") for h in range(2)]
                    for h in range(2):
                        diag = bass.AP(
                            tensor=we[h][:].tensor, offset=127,
                            ap=[[WSTR, 128], [1, N]])
                        nc.sync.dma_start(out=rsh[h][:], in_=diag)
                    rsh_q[ci] = rsh

                def content_stage(ci):
                    # content logits + rel add + exp + transpose
                    s, g = ci // 4, ci % 4
                    i0 = 128 * ci
                    rsh = rsh_q.pop(ci)
                    if g == 0:
                        ptT_q[s] = [pt_pool.tile([128, PTW], BF16, name=f"ptT{h}",
                                                 tag=f"ptT{h}") for h in range(2)]
                    ptT = ptT_q[s]
                    p_t = [p_pool.tile([128, N], BF16, name=f"p{h}",
                                       tag=f"p{h}") for h in range(2)]
                    for jb in range(3):
                        j0 = 512 * jb
                        pc = [ps_c.tile([128, 512], F32, name="pc", tag="pc")
                              for _ in range(2)]
                        for h in range(2):
                            hs = slice(64 * h, 64 * h + 64)
                            nc.tensor.matmul(pc[h][:], qc_t[hs, i0:i0 + 128],
                                             k_t[hs, j0:j0 + 512],
                                             start=True, stop=False)
                        for h in range(2):
                            nc.tensor.matmul(pc[h][:], ident_b[:],
                                             rsh[h][:, j0:j0 + 512],
                                             start=False, stop=True)
                        for h in range(2):
                            nc.scalar.activation(
                                p_t[h][:, j0:j0 + 512], pc[h][:],
                                mybir.ActivationFunctionType.Exp)
                    for h in range(2):
                        outap = bass.AP(
                            tensor=ptT[h][:].tensor, offset=128 * g,
                            ap=[[PTW, 128], [512, NCH], [1, 128]])
                        nc.sync.dma_start(out=outap, in_=p_t[h][:],
                                          transpose=True)

                def po_stage(s):
                    # attn @ v with ones-column row sums, then normalize
                    cs = slice(512 * s, 512 * s + 512)
                    ptT = ptT_q.pop(s)
                    for h in range(2):
                        po1 = ps_o.tile([128, 512], F32, name=f"po1{h}",
                                        tag="po1")
                        po2 = ps_o.tile([65, 512], F32, name=f"po2{h}",
                                        tag="po2")
                        vo1 = 193 * h
                        vo2 = 193 * h + 128
                        for jb in range(NCH):
                            nc.tensor.matmul(po1[:], v_t[jb][:, vo1:vo1 + 128],
                                             ptT[h][:, 512 * jb:512 * jb + 512],
                                             start=(jb == 0), stop=(jb == NCH - 1))
                        for jb in range(NCH):
                            nc.tensor.matmul(po2[:], v_t[jb][:, vo2:vo2 + 65],
                                             ptT[h][:, 512 * jb:512 * jb + 512],
                                             start=(jb == 0), stop=(jb == NCH - 1))
                        rrow = small.tile([1, 512], F32, name=f"rr{h}",
                                          tag=f"rr{h}")
                        nc.vector.reciprocal(rrow[:], po2[64:65, :])
                        brow = small.tile([128, 512], F32, name=f"br{h}",
                                          tag=f"br{h}")
                        nc.gpsimd.partition_broadcast(brow[:], rrow[:])
                        nc.vector.tensor_mul(o1_h[h][:, cs], po1[:], brow[:])
                        nc.vector.tensor_mul(o2s[64 * h:64 * h + 64, cs],
                                             po2[0:64, :], brow[0:64, :])

                def outproj_stage(s):
                    # project this super's finished o columns; pf buffers
                    # rotate through the freed ps_o banks.
                    for r in range(4 * s, 4 * s + 4):
                        i0 = 128 * r
                        for ob in range(3):
                            c0 = 512 * ob
                            pf = ps_o.tile([128, 512], F32, name="pf",
                                           tag=po_tags[(3 * r + ob) % 2])
                            for kc in range(3):
                                nc.tensor.matmul(
                                    pf[:], o_pieces[kc][:, i0:i0 + 128],
                                    wo_t[kc][:, c0:c0 + 512],
                                    start=(kc == 0), stop=(kc == 2))
                            osb = osb_pool.tile([128, 512], BF16, name="osb",
                                                tag="osb")
                            if ob % 2 == 0:
                                nc.scalar.copy(osb[:], pf[:])
                            else:
                                nc.vector.tensor_copy(osb[:], pf[:])
                            nc.scalar.dma_start(
                                out=out_d[i0:i0 + 128, c0:c0 + 512], in_=osb[:])

                # software-pipelined schedule: wwin runs 2 chunks ahead of
                # content; po for super s issues after the first chunk of
                # super s+1 so the transpose DMAs have time to drain; the
                # output projection of super s follows its po immediately.
                wwin_stage(0)
                wwin_stage(1)
                for ci in range(4 * NSUP):
                    if ci + 2 < 4 * NSUP:
                        wwin_stage(ci + 2)
                    content_stage(ci)
                    if ci % 4 == 1 and ci > 4:
                        po_stage(ci // 4 - 1)
                        outproj_stage(ci // 4 - 1)
                po_stage(NSUP - 1)
                outproj_stage(NSUP - 1)

    nc.compile()
    return nc


_NC_CACHE = None


def _get_nc():
    global _NC_CACHE
    if _NC_CACHE is None:
        _NC_CACHE = build_nc()
    return _NC_CACHE


def _get_positional_embed_np(n, feature_size):
    """numpy mirror of the reference's jax positional embedding (float64)."""
    from scipy.special import gammaln, xlogy

    nb = feature_size // 6
    dist = np.arange(-n + 1, n, dtype=np.float64)
    ad = np.abs(dist)[:, None]

    max_range = math.log(n) / math.log(2.0)
    half_life = 2.0 ** np.linspace(3.0, max_range, nb)
    f_exp = np.exp(-math.log(2.0) / half_life[None, :] * ad)

    center_widths = 2.0 ** np.arange(1, nb + 1, dtype=np.float64) - 1.0
    f_cm = (center_widths[None, :] > ad).astype(np.float64)

    stddev = n / (2.0 * nb)
    start_mean = n / nb
    mean = np.linspace(start_mean, float(n), nb)[None, :]
    concentration = (mean / stddev) ** 2
    rate = mean / (stddev**2)
    log_unnorm = xlogy(concentration - 1.0, ad) - rate * ad
    log_norm = gammaln(concentration) - concentration * np.log(rate)
    probs = np.exp(log_unnorm - log_norm) + 1e-8
    f_g = probs / np.max(probs)

    emb = np.concatenate([f_exp, f_cm, f_g], axis=-1)
    return np.concatenate([emb, np.sign(dist)[:, None] * emb], axis=-1)


_POST_CACHE = None


def _get_posT():
    global _POST_CACHE
    if _POST_CACHE is None:
        p = _get_positional_embed_np(N, NRPF).T
        full = np.zeros((NRPF, POSW), np.float32)
        full[:, :TWO_N1] = p
        _POST_CACHE = full.astype(ml_dtypes.bfloat16)
    return _POST_CACHE


def make_in_maps(inputs):
    x = np.asarray(inputs["x"], np.float32)
    Wq = np.asarray(inputs["Wq"], np.float32)
    Wk = np.asarray(inputs["Wk"], np.float32)
    Wv = np.asarray(inputs["Wv"], np.float32)
    W_rel_k = np.asarray(inputs["W_rel_k"], np.float32).astype(ml_dtypes.bfloat16)
    bc = np.asarray(inputs["rel_content_bias"], np.float32)[0, :, 0, :]  # (H, DK)
    bp = np.asarray(inputs["rel_pos_bias"], np.float32)[0, :, 0, :]
    Wo = np.asarray(inputs["Wo"], np.float32).astype(ml_dtypes.bfloat16)
    posT = _get_posT()
    xT = [np.ascontiguousarray(x[b].T) for b in range(2)]
    in_maps = []
    for core in range(8):
        b, hp = core // 4, core % 4
        in_maps.append({
            "xt_b": xT[b],
            "wq_s": np.ascontiguousarray(Wq[:, 128 * hp:128 * hp + 128]),
            "wk_s": np.ascontiguousarray(Wk[:, 128 * hp:128 * hp + 128]),
            "wv_s": np.ascontiguousarray(Wv[:, 384 * hp:384 * hp + 384]),
            "wrk_s": np.ascontiguousarray(W_rel_k[:, 128 * hp:128 * hp + 128]),
            "posT": posT,
            "bc_s": np.ascontiguousarray(bc[2 * hp:2 * hp + 2].reshape(128)),
            "bp_s": np.ascontiguousarray(bp[2 * hp:2 * hp + 2].reshape(128)),
            "wo_s": np.ascontiguousarray(np.concatenate([
                Wo[384 * hp:384 * hp + 128],
                Wo[384 * hp + 192:384 * hp + 320],
                Wo[384 * hp + 128:384 * hp + 192],
                Wo[384 * hp + 320:384 * hp + 384]], axis=0)),
        })
    return in_maps


def kernel(x, Wq, Wk, Wv, W_rel_k, rel_content_bias, rel_pos_bias, Wo, bo):
    bo = np.asarray(bo, np.float32)
    in_maps = make_in_maps(dict(
        x=x, Wq=Wq, Wk=Wk, Wv=Wv, W_rel_k=W_rel_k,
        rel_content_bias=rel_content_bias, rel_pos_bias=rel_pos_bias, Wo=Wo))
    nc = _get_nc()
    res = run_bass_kernel_spmd(nc, in_maps, list(range(8)))
    out = np.zeros((2, N, DIM), np.float32)
    for core in range(8):
        out[core // 4] += res.results[core]["out_p"].astype(np.float32)
    out += bo
    return out
